# revision 59
# baseline (speedup 1.0000x reference)
"""Trainium2 Bass kernel for single-head causal attention (fp8 version).

  q = Xq @ Wq.T + bq ; k = Xk @ Wk.T + bk ; v = Xv @ Wv.T + bv
  out = softmax((q k^T + causal_mask)/sqrt(D)) @ v

Shapes: B=4, S=2048, D=1024, fp32 in/out.  8 NeuronCores, SPMD.

Sharding (uniform causal schedule -> identical program on every core):
  core c handles batch b = c//2, parity h = c%2.
  Within the batch, S splits into 16 q-tiles of 128.  q-tile g needs
  ceil((g+1)/4) k-chunks of 512.  Core (b, h) takes, for every chunk-count
  class n in {1,2,3,4}, the two tiles g = 4*(n-1) + 2*h and +2*h+1.  Each
  core owns 8 q-tiles with chunk counts [1,1,2,2,3,3,4,4].

Precision plan (rel-err budget 2e-2; emulated max err ~7e-3):
  - Q/K/V projections run in fp8e4 (e4m3) with DoubleRow perf mode
    (0.5 PE cycles/row): weights pre-scaled x256 (Wq/Wk; /256 folded into
    the bias activation) or x64 (Wv; /64 folded into the softmax
    denominator reciprocal).
  - scores (q k^T) and attn@V also fp8 DoubleRow; P transposed on the PE.
  - K^T / V / Q^T all stay SBUF-resident (fp8: 16+16+8 KB/partition).
  - few-key softmax rows (0..127) are precision-critical: a small bf16
    side path recomputes q/k/v tiles for rows/keys<128 and produces the
    g-tile-0 output (used by h=0 cores; ignored by h=1).
"""

from contextlib import ExitStack

import numpy as np
import ml_dtypes

import concourse.bacc as bacc
import concourse.mybir as mybir
import concourse.tile as tile
from concourse.bass_utils import run_bass_kernel_spmd
from concourse.masks import make_identity

P = 128
D = 1024
S = 2048
B = 4
N_CORES = 8
EO = D // P            # 8 contraction subtiles of 128
EP = EO // 2           # 4 DoubleRow pairs
DO = D // P            # 8 output-dim subtiles
KC = S // 512          # 4 k-chunks of 512
NQ = 8                 # q-slots per core
# slot j -> (n_chunks, pair_idx); global q-tile g = 4*(n-1) + 2*h + p
SLOTS = [(1, 0), (1, 1), (2, 0), (2, 1), (3, 0), (3, 1), (4, 0), (4, 1)]
F32 = mybir.dt.float32
F8 = mybir.dt.float8e4
BF16 = mybir.dt.bfloat16
NEG = -1.0e9
WQK_SCALE = 256.0      # Wq/Wk fp8 pre-scale (undone in the bias activation)
WV_SCALE = 32.0        # Wv fp8 pre-scale (undone in the denominator);
                       # kept low: device fp8e4 saturates to non-finite
                       # above ~240, and |v| can reach ~4.1
# The on-device f32->fp8 casts truncate (round toward zero).  Pre-scaling a
# value by ~half a relative ulp before the cast re-centers the quantization
# error; the factor is folded into activation scales / the denominator.
RTZ_C = 1.046875
NP_F8 = ml_dtypes.float8_e4m3fn
NP_BF16 = ml_dtypes.bfloat16

_PROG_CACHE = {}


def _slot_gtiles(h, causal):
    if causal:
        return [4 * (n - 1) + 2 * h + p for (n, p) in SLOTS]
    return [8 * h + j for j in range(NQ)]


def build_program(causal: bool):
    nc = bacc.Bacc(trn_type="TRN2", target_bir_lowering=False, debug=False)

    def din(name, shape, dt=F32):
        return nc.dram_tensor(name, shape, dt, kind="ExternalInput").ap()

    xq = din("xq", [2, P, EO, 512], F8)
    xk = din("xk", [KC, P, EO, 512], F8)
    xv = din("xv", [16, P, EO, P], F8)
    wq = din("wq", [P, EO, D], F8)
    wk = din("wk", [P, EO, D], F8)
    wv = din("wv", [P, EO, D], F8)
    bq = din("bq", [P, DO])
    bk = din("bk", [P, DO])
    bq2 = din("bq2", [P, DO])      # bq * RTZ_C, for the fp8 projection path
    bk2 = din("bk2", [P, DO])
    bv = din("bv", [P, D])
    cm = din("cm", [P, 2, 512])
    if causal:
        xq0 = din("xq0", [P, EO, P], BF16)
        xk0 = din("xk0", [P, EO, P], BF16)
        xv0 = din("xv0", [P, EO, P], BF16)
        wqb = din("wqb", [P, EO, D], BF16)
        wkb = din("wkb", [P, EO, D], BF16)
        wvb = din("wvb", [P, EO, D], BF16)
        cm0 = din("cm0", [P, P])
    out = nc.dram_tensor("out", [NQ, P, D], F32, kind="ExternalOutput").ap()
    if causal:
        outg0 = nc.dram_tensor("outg0", [P, D], F32, kind="ExternalOutput").ap()

    Ident = mybir.ActivationFunctionType.Identity
    Exp = mybir.ActivationFunctionType.Exp
    Copy = mybir.ActivationFunctionType.Copy
    add = mybir.AluOpType.add
    mult = mybir.AluOpType.mult
    DR = mybir.MatmulPerfMode.DoubleRow

    nchunks = [n for (n, _) in SLOTS] if causal else [KC] * NQ
    inv_s = float(1.0 / np.sqrt(D))
    inv_wqk2 = float(RTZ_C / WQK_SCALE)
    exp_scale = float(1.0 / (np.sqrt(D) * RTZ_C * RTZ_C))
    exp_bias = float(np.log(RTZ_C))

    with tile.TileContext(nc, pool_alloc_mode="queue") as tc, ExitStack() as top:
        const = top.enter_context(tc.tile_pool(name="const", bufs=1))
        ident8 = const.tile([P, P], F8)
        identb = const.tile([P, P], BF16)
        ident32 = const.tile([P, P], F32)
        make_identity(nc, ident32)
        nc.vector.tensor_copy(out=ident8, in_=ident32)
        nc.vector.tensor_copy(out=identb, in_=ident32)
        bq_sb = const.tile([P, DO], F32)
        nc.gpsimd.dma_start(out=bq_sb, in_=bq)
        bk_sb = const.tile([P, DO], F32)
        nc.gpsimd.dma_start(out=bk_sb, in_=bk)
        bq2_sb = const.tile([P, DO], F32)
        nc.gpsimd.dma_start(out=bq2_sb, in_=bq2)
        bk2_sb = const.tile([P, DO], F32)
        nc.gpsimd.dma_start(out=bk2_sb, in_=bk2)
        bv_sb = const.tile([P, D], F32)
        nc.gpsimd.dma_start(out=bv_sb, in_=bv)
        cm_sb = const.tile([P, 2, 512], F32)
        ebias_sb = const.tile([P, 1], F32)
        nc.gpsimd.memset(ebias_sb, exp_bias)
        if causal:
            cm0_sb = const.tile([P, P], F32)

        # persistent SBUF-resident activations
        res = top.enter_context(tc.tile_pool(name="res", bufs=1))
        qt_sb = res.tile([P, DO, 1024], F8, name="qt_sb")     # q^T
        kt_sb = res.tile([P, DO, 2048], F8, name="kt_sb")     # k^T
        v_sb = res.tile([P, 16, D], F8, name="v_sb")          # v (x WV_SCALE)
        if causal:
            qt0_sb = res.tile([P, DO, P], BF16, name="qt0_sb")
            kt0_sb = res.tile([P, DO, P], BF16, name="kt0_sb")
            v0_sb = res.tile([P, D], BF16, name="v0_sb")
        accp = top.enter_context(tc.tile_pool(name="accp", bufs=1))
        accs = [accp.tile([P, KC], F32, name=f"acc{j}") for j in range(NQ)]
        if causal:
            accg = accp.tile([P, 1], F32, name="accg")

        # ---------------- projections ----------------
        # All input-DMA triggers are emitted up front so the two hardware
        # DGE rings (sync / scalar) start streaming at t=0 and no trigger
        # gets stuck behind activations in the scalar engine's queue.
        # Every input tile is a distinct pool slot (no WAR deps on triggers).
        with nc.named_scope("proj"), \
             tc.tile_pool(name="wt8", bufs=3) as wt8, \
             tc.tile_pool(name="wtb", bufs=3) as wtb, \
             tc.tile_pool(name="xin", bufs=6) as xinp, \
             tc.tile_pool(name="xvp", bufs=16) as xvp, \
             tc.tile_pool(name="xg0", bufs=3) as xg0, \
             tc.tile_pool(name="psA", bufs=4, space="PSUM") as psA, \
             tc.tile_pool(name="psB", bufs=2, space="PSUM") as psB:

            wk_sb = wt8.tile([P, EO, D], F8, tag="wt8", name="wk_sb")
            wq_sb = wt8.tile([P, EO, D], F8, tag="wt8", name="wq_sb")
            wv_sb = wt8.tile([P, EO, D], F8, tag="wt8", name="wv_sb")
            xk_ts = [xinp.tile([P, EO, 512], F8, tag="xin", name=f"xk_t{kc}")
                     for kc in range(KC)]
            xq_ts = [xinp.tile([P, EO, 512], F8, tag="xin", name=f"xq_t{sc}")
                     for sc in range(2)]
            xv_ts = [xvp.tile([P, EO, P], F8, tag="xv", name=f"xv_t{st}")
                     for st in range(16)]
            if causal:
                wqb_sb = wtb.tile([P, EO, D], BF16, tag="wtb", name="wqb_sb")
                wkb_sb = wtb.tile([P, EO, D], BF16, tag="wtb", name="wkb_sb")
                wvb_sb = wtb.tile([P, EO, D], BF16, tag="wtb", name="wvb_sb")
                xq0_t = xg0.tile([P, EO, P], BF16, tag="xg0", name="xq0_t")
                xk0_t = xg0.tile([P, EO, P], BF16, tag="xg0", name="xk0_t")
                xv0_t = xg0.tile([P, EO, P], BF16, tag="xg0", name="xv0_t")

            # DMA trigger instructions cost ~0.6-0.8us on the issuing engine
            # and BLOCK it when the hardware ring backs up.  The scalar
            # engine (which must run the projection activations from ~12us)
            # therefore issues only the 8 earliest triggers; the sync and
            # gpsimd engines, which have no compute, carry everything else.
            # scalar ring: K-chunk-0 + Q-slab-0 inputs only.  Whole-tile
            # transfers: contiguous 4-8KB partition lines run ~4x faster
            # than the 1-2KB strided per-pair slices.
            nc.scalar.dma_start(out=xk_ts[0], in_=xk[0])
            nc.scalar.dma_start(out=xq_ts[0], in_=xq[0])
            nc.scalar.dma_start(out=wq_sb, in_=wq)
            # sync ring: wk first (K0 critical path), then inputs in
            # consumption order, bf16 g0 weights last
            nc.sync.dma_start(out=wk_sb, in_=wk)
            nc.sync.dma_start(out=xq_ts[1], in_=xq[1])
            nc.sync.dma_start(out=wv_sb, in_=wv)
            nc.sync.dma_start(out=xk_ts[1], in_=xk[1])
            nc.sync.dma_start(out=xk_ts[2], in_=xk[2])
            nc.sync.dma_start(out=xk_ts[3], in_=xk[3])
            if causal:
                nc.sync.dma_start(out=wqb_sb, in_=wqb)
            # gpsimd (software DGE, ~135 GB/s measured): v tiles, g0
            # inputs, and the two late bf16 weights
            for st in range(16):
                nc.gpsimd.dma_start(out=xv_ts[st], in_=xv[st])
            nc.gpsimd.dma_start(out=cm_sb, in_=cm)
            if causal:
                nc.gpsimd.dma_start(out=cm0_sb, in_=cm0)
                nc.gpsimd.dma_start(out=xq0_t, in_=xq0)
                nc.gpsimd.dma_start(out=xk0_t, in_=xk0)
                nc.gpsimd.dma_start(out=xv0_t, in_=xv0)
                nc.gpsimd.dma_start(out=wkb_sb, in_=wkb)
                nc.gpsimd.dma_start(out=wvb_sb, in_=wvb)

            def kq_chunk(w_sb, x_t, kt_dst, b_sb):
                """one 512-col slab of the K or Q projection (fp8 DoubleRow).

                The psum->fp8 drain alternates scalar/vector so psum slot
                recycling is not bound by a single engine's latency."""
                for do in range(DO):
                    ps = psA.tile([P, 512], F32, tag="psA", name="pskq")
                    for ep in range(EP):
                        nc.tensor.matmul(
                            ps,
                            lhsT=w_sb[:, 2 * ep:2 * ep + 2, do * P:(do + 1) * P],
                            rhs=x_t[:, 2 * ep:2 * ep + 2, :],
                            start=(ep == 0), stop=(ep == EP - 1),
                            perf_mode=DR)
                    if do % 2 == 0:
                        nc.scalar.activation(out=kt_dst[:, do, :], in_=ps,
                                             func=Ident, scale=inv_wqk2,
                                             bias=b_sb[:, do:do + 1])
                    else:
                        nc.vector.tensor_scalar(
                            out=kt_dst[:, do, :], in0=ps,
                            scalar1=inv_wqk2, scalar2=b_sb[:, do:do + 1],
                            op0=mult, op1=add)

            def v_tile(st):
                xv_t = xv_ts[st]
                ps2 = psB.tile([P, D], F32, tag="psB", name=f"psv{st}")
                for half in range(2):
                    for ep in range(EP):
                        nc.tensor.matmul(
                            ps2[:, half * 512:(half + 1) * 512],
                            lhsT=xv_t[:, 2 * ep:2 * ep + 2, :],
                            rhs=wv_sb[:, 2 * ep:2 * ep + 2,
                                      half * 512:(half + 1) * 512],
                            start=(ep == 0), stop=(ep == EP - 1),
                            perf_mode=DR)
                if st % 2 == 0:
                    nc.vector.tensor_scalar_mul(v_sb[:, st, :], ps2, RTZ_C)
                else:
                    nc.scalar.activation(out=v_sb[:, st, :], in_=ps2,
                                         func=Copy, scale=RTZ_C)

            def kq_chunk_eo(w_sb, x_t, kt_dst, b_sb):
                """ep-outer variant for the startup phases: weight pairs are
                consumed incrementally as their DMAs land, instead of every
                do-block stalling on the full weight tile."""
                for half in range(2):
                    dos = range(half * 4, half * 4 + 4)
                    pss = [psA.tile([P, 512], F32, tag="psA",
                                    name=f"pseo{half}_{i}")
                           for i in range(4)]
                    for ep in range(EP):
                        for i, do in enumerate(dos):
                            nc.tensor.matmul(
                                pss[i],
                                lhsT=w_sb[:, 2 * ep:2 * ep + 2,
                                          do * P:(do + 1) * P],
                                rhs=x_t[:, 2 * ep:2 * ep + 2, :],
                                start=(ep == 0), stop=(ep == EP - 1),
                                perf_mode=DR)
                    for i, do in enumerate(dos):
                        if do % 2 == 0:
                            nc.scalar.activation(out=kt_dst[:, do, :],
                                                 in_=pss[i], func=Ident,
                                                 scale=inv_wqk2,
                                                 bias=b_sb[:, do:do + 1])
                        else:
                            nc.vector.tensor_scalar(
                                out=kt_dst[:, do, :], in0=pss[i],
                                scalar1=inv_wqk2, scalar2=b_sb[:, do:do + 1],
                                op0=mult, op1=add)

            # K chunk 0 first (earliest attention dependency)
            kq_chunk_eo(wk_sb, xk_ts[0], kt_sb[:, :, 0:512], bk2_sb)

            # Q projection (both slabs)
            kq_chunk_eo(wq_sb, xq_ts[0], qt_sb[:, :, 0:512], bq2_sb)
            kq_chunk(wq_sb, xq_ts[1], qt_sb[:, :, 512:1024], bq2_sb)

            # V tiles interleaved with the remaining K chunks; the g0 bf16
            # side path runs last, by which time its weights (low-priority
            # tail of both hardware rings) have arrived.
            for st in range(4):
                v_tile(st)
            kq_chunk(wk_sb, xk_ts[1], kt_sb[:, :, 512:1024], bk2_sb)
            for st in range(4, 8):
                v_tile(st)
            for kc in range(2, KC):
                kq_chunk(wk_sb, xk_ts[kc],
                         kt_sb[:, :, kc * 512:(kc + 1) * 512], bk2_sb)
                for st in range(4 * kc, 4 * kc + 4):
                    v_tile(st)

            if causal:
                for (wb, xt, dst, bb) in ((wqb_sb, xq0_t, qt0_sb, bq_sb),
                                          (wkb_sb, xk0_t, kt0_sb, bk_sb)):
                    for do in range(DO):
                        psg = psA.tile([P, P], F32, tag="psA", name="psg")
                        for eo in range(EO):
                            nc.tensor.matmul(
                                psg,
                                lhsT=wb[:, eo, do * P:(do + 1) * P],
                                rhs=xt[:, eo, :],
                                start=(eo == 0), stop=(eo == EO - 1))
                        nc.scalar.activation(out=dst[:, do, :], in_=psg,
                                             func=Ident,
                                             bias=bb[:, do:do + 1])
                psv0 = psB.tile([P, D], F32, tag="psB", name="psv0")
                for half in range(2):
                    for eo in range(EO):
                        nc.tensor.matmul(
                            psv0[:, half * 512:(half + 1) * 512],
                            lhsT=xv0_t[:, eo, :],
                            rhs=wvb_sb[:, eo, half * 512:(half + 1) * 512],
                            start=(eo == 0), stop=(eo == EO - 1))
                nc.vector.tensor_copy(out=v0_sb, in_=psv0)

        # ---------------- attention, q-major, software-pipelined ----------
        with nc.named_scope("attn"), \
             tc.tile_pool(name="pep", bufs=3) as pep, \
             tc.tile_pool(name="ptp", bufs=3) as ptp, \
             tc.tile_pool(name="denp", bufs=4) as denp, \
             tc.tile_pool(name="outp", bufs=3) as outp, \
             tc.tile_pool(name="psS", bufs=2, space="PSUM") as psS, \
             tc.tile_pool(name="psT", bufs=2, space="PSUM") as psT, \
             tc.tile_pool(name="psV", bufs=2, space="PSUM") as psV:

            items = [(j, c) for j in range(NQ) for c in range(nchunks[j])]
            units = (["g0"] if causal else []) + items
            state = {}
            av_of = {}

            def emit_S(u):
                if u == "g0":
                    psg = psS.tile([P, P], F32, tag="s", name="psg0")
                    for do in range(DO):
                        nc.tensor.matmul(
                            psg, lhsT=qt0_sb[:, do, :], rhs=kt0_sb[:, do, :],
                            start=(do == 0), stop=(do == DO - 1))
                    nc.vector.tensor_tensor(out=psg, in0=psg, in1=cm0_sb, op=add)
                    peg = pep.tile([P, P], BF16, tag="p", name="peg0")
                    nc.scalar.activation(out=peg, in_=psg, func=Exp,
                                         scale=inv_s, accum_out=accg)
                    state[u] = peg
                    return
                j, c = u
                ps = psS.tile([P, 512], F32, tag="s", name=f"ps{c}_{j}")
                for ep in range(EP):
                    nc.tensor.matmul(
                        ps,
                        lhsT=qt_sb[:, 2 * ep:2 * ep + 2, j * P:(j + 1) * P],
                        rhs=kt_sb[:, 2 * ep:2 * ep + 2, c * 512:(c + 1) * 512],
                        start=(ep == 0), stop=(ep == EP - 1),
                        perf_mode=DR)
                if causal and c == nchunks[j] - 1:
                    p_j = SLOTS[j][1]
                    nc.vector.tensor_tensor(out=ps, in0=ps,
                                            in1=cm_sb[:, p_j, :], op=add)
                pe = pep.tile([P, 512], BF16, tag="p", name=f"pe{c}_{j}")
                nc.scalar.activation(out=pe, in_=ps, func=Exp,
                                     scale=exp_scale, bias=ebias_sb,
                                     accum_out=accs[j][:, c:c + 1])
                state[u] = pe

            def emit_T(u):
                pe = state[u]
                if u == "g0":
                    ptps = psT.tile([P, P], BF16, tag="t", name="ptg0")
                    nc.tensor.transpose(ptps, pe, identb)
                    pt = ptp.tile([P, P], BF16, tag="pt", name="ptg0_sb")
                    nc.vector.tensor_copy(out=pt, in_=ptps)
                else:
                    j, c = u
                    ptps = psT.tile([P, 4, P], BF16, tag="t", name=f"ptps{c}_{j}")
                    for t in range(4):
                        nc.tensor.transpose(ptps[:, t, :],
                                            pe[:, t * P:(t + 1) * P], identb)
                    pt = ptp.tile([P, 4, P], F8, tag="pt", name=f"pt{c}_{j}")
                    nc.vector.tensor_copy(out=pt, in_=ptps)
                state[u] = pt

            def emit_A(u):
                pt = state.pop(u)
                if u == "g0":
                    avg = psV.tile([P, D], F32, tag="avp", name="avg0")
                    for half in range(2):
                        nc.tensor.matmul(
                            avg[:, half * 512:(half + 1) * 512],
                            lhsT=pt,
                            rhs=v0_sb[:, half * 512:(half + 1) * 512],
                            start=True, stop=True)
                    recg = denp.tile([P, 1], F32, tag="rec", name="recg0")
                    nc.vector.reciprocal(out=recg, in_=accg)
                    og = outp.tile([P, D], F32, tag="o", name="og0")
                    nc.vector.scalar_tensor_tensor(
                        out=og, in0=avg, scalar=recg, in1=bv_sb,
                        op0=mult, op1=add)
                    nc.sync.dma_start(out=outg0, in_=og)
                    return
                j, c = u
                n_j = nchunks[j]
                if c == 0:
                    av_of[j] = psV.tile([P, D], F32, tag="avp", name=f"av{j}")
                av = av_of[j]
                for tp in range(2):
                    for half in range(2):
                        nc.tensor.matmul(
                            av[:, half * 512:(half + 1) * 512],
                            lhsT=pt[:, 2 * tp:2 * tp + 2, :],
                            rhs=v_sb[:, 4 * c + 2 * tp:4 * c + 2 * tp + 2,
                                     half * 512:(half + 1) * 512],
                            start=(c == 0 and tp == 0),
                            stop=(c == n_j - 1 and tp == 1),
                            perf_mode=DR)
                if c == n_j - 1:
                    den = denp.tile([P, 1], F32, tag="den", name=f"den{j}")
                    nc.vector.tensor_reduce(
                        out=den, in_=accs[j][:, 0:n_j],
                        axis=mybir.AxisListType.X, op=add)
                    den64 = denp.tile([P, 1], F32, tag="den64", name=f"den64{j}")
                    nc.vector.tensor_scalar_mul(den64, den, float(WV_SCALE * RTZ_C))
                    rec = denp.tile([P, 1], F32, tag="rec", name=f"rec{j}")
                    nc.vector.reciprocal(out=rec, in_=den64)
                    o = outp.tile([P, D], F32, tag="o", name=f"o{j}")
                    nc.vector.scalar_tensor_tensor(
                        out=o, in0=av_of.pop(j), scalar=rec, in1=bv_sb,
                        op0=mult, op1=add)
                    nc.sync.dma_start(out=out[j], in_=o)

            N = len(units)
            for t in range(N + 2):
                if t < N:
                    emit_S(units[t])
                if 1 <= t <= N:
                    emit_T(units[t - 1])
                if t >= 2:
                    emit_A(units[t - 2])

    nc.compile()
    return nc


def _get_program(causal: bool):
    key = bool(causal)
    if key not in _PROG_CACHE:
        _PROG_CACHE[key] = build_program(key)
    return _PROG_CACHE[key]


def _shard_inputs(encoded_q, encoded_k, encoded_v, W_q, b_q, W_k, b_k,
                  W_v, b_v, causal):
    """Build the per-core in_maps (all host-side numpy)."""
    def wlayout(W, scale, dt):
        return np.ascontiguousarray(
            (W.T * scale).reshape(EO, P, D).transpose(1, 0, 2)).astype(dt)

    wq8 = wlayout(W_q, WQK_SCALE, NP_F8)
    wk8 = wlayout(W_k, WQK_SCALE, NP_F8)
    wv8 = wlayout(W_v, WV_SCALE, NP_F8)
    bqh = np.ascontiguousarray(b_q.reshape(DO, P).T)
    bkh = np.ascontiguousarray(b_k.reshape(DO, P).T)
    bvh = np.ascontiguousarray(np.broadcast_to(b_v, (P, D)))
    if causal:
        wqb = wlayout(W_q, 1.0, NP_BF16)
        wkb = wlayout(W_k, 1.0, NP_BF16)
        wvb = wlayout(W_v, 1.0, NP_BF16)
        qi = np.arange(P)[:, None]
        cm0h = np.where(np.arange(P)[None, :] <= qi, 0.0, NEG).astype(np.float32)

    kf = np.arange(512)[None, :]
    in_maps = []
    for c in range(N_CORES):
        b, h = divmod(c, 2)
        gts = _slot_gtiles(h, causal)
        Xq = np.concatenate([encoded_q[b, g * P:(g + 1) * P, :] for g in gts], 0)
        xqh = np.ascontiguousarray(
            Xq.T.reshape(EO, P, 2, 512).transpose(2, 1, 0, 3)).astype(NP_F8)
        xkh = np.ascontiguousarray(
            encoded_k[b].T.reshape(EO, P, KC, 512).transpose(2, 1, 0, 3)
        ).astype(NP_F8)
        xvh = np.ascontiguousarray(
            encoded_v[b].T.reshape(EO, P, 16, P).transpose(2, 1, 0, 3)
        ).astype(NP_F8)
        cmh = np.zeros((P, 2, 512), np.float32)
        im = {
            "xq": xqh, "xk": xkh, "xv": xvh,
            "wq": wq8, "wk": wk8, "wv": wv8,
            "bq": bqh, "bk": bkh, "bv": bvh, "cm": cmh,
            "bq2": np.float32(RTZ_C) * bqh, "bk2": np.float32(RTZ_C) * bkh,
        }
        if causal:
            qi = np.arange(P)[:, None]
            for p in range(2):
                r = 2 * h + p
                cmh[:, p, :] = np.where(kf <= r * P + qi, 0.0, NEG)
            def x0layout(X):
                return np.ascontiguousarray(
                    X[b, :P, :].T.reshape(EO, P, P).transpose(1, 0, 2)
                ).astype(NP_BF16)
            im.update({
                "xq0": x0layout(encoded_q), "xk0": x0layout(encoded_k),
                "xv0": x0layout(encoded_v),
                "wqb": wqb, "wkb": wkb, "wvb": wvb, "cm0": cm0h,
            })
        in_maps.append(im)
    return in_maps


def kernel(encoded_q, encoded_k, encoded_v, W_q, b_q, W_k, b_k, W_v, b_v,
           parameter_mask, _want_trace=False, _trace_dir=None):
    causal = bool(np.asarray(parameter_mask).item())
    encoded_q = np.asarray(encoded_q, np.float32)
    encoded_k = np.asarray(encoded_k, np.float32)
    encoded_v = np.asarray(encoded_v, np.float32)
    nc = _get_program(causal)
    in_maps = _shard_inputs(encoded_q, encoded_k, encoded_v,
                            np.asarray(W_q, np.float32), np.asarray(b_q, np.float32),
                            np.asarray(W_k, np.float32), np.asarray(b_k, np.float32),
                            np.asarray(W_v, np.float32), np.asarray(b_v, np.float32),
                            causal)
    kw = {}
    if _want_trace:
        kw = dict(trace=True, tmpdir=_trace_dir)
    res = run_bass_kernel_spmd(nc, in_maps, core_ids=list(range(N_CORES)), **kw)

    full = np.empty((B, S, D), np.float32)
    for c in range(N_CORES):
        b, h = divmod(c, 2)
        o = res.results[c]["out"]
        for j, g in enumerate(_slot_gtiles(h, causal)):
            full[b, g * P:(g + 1) * P, :] = o[j]
        if causal and h == 0:
            full[b, 0:P, :] = res.results[c]["outg0"]
    if _want_trace:
        return full, res
    return full


# revision 61
# speedup vs baseline: 1.0527x; 1.0527x over previous
"""Trainium2 Bass kernel for single-head causal attention (fp8 version).

  q = Xq @ Wq.T + bq ; k = Xk @ Wk.T + bk ; v = Xv @ Wv.T + bv
  out = softmax((q k^T + causal_mask)/sqrt(D)) @ v

Shapes: B=4, S=2048, D=1024, fp32 in/out.  8 NeuronCores, SPMD.

Sharding (uniform causal schedule -> identical program on every core):
  core c handles batch b = c//2, parity h = c%2.
  Within the batch, S splits into 16 q-tiles of 128.  q-tile g needs
  ceil((g+1)/4) k-chunks of 512.  Core (b, h) takes, for every chunk-count
  class n in {1,2,3,4}, the two tiles g = 4*(n-1) + 2*h and +2*h+1.  Each
  core owns 8 q-tiles with chunk counts [1,1,2,2,3,3,4,4].

Precision plan (rel-err budget 2e-2; emulated max err ~7e-3):
  - Q/K/V projections run in fp8e4 (e4m3) with DoubleRow perf mode
    (0.5 PE cycles/row): weights pre-scaled x256 (Wq/Wk; /256 folded into
    the bias activation) or x64 (Wv; /64 folded into the softmax
    denominator reciprocal).
  - scores (q k^T) and attn@V also fp8 DoubleRow; P transposed on the PE.
  - K^T / V / Q^T all stay SBUF-resident (fp8: 16+16+8 KB/partition).
  - few-key softmax rows (0..127) are precision-critical: a small bf16
    side path recomputes q/k/v tiles for rows/keys<128 and produces the
    g-tile-0 output (used by h=0 cores; ignored by h=1).
"""

from contextlib import ExitStack

import numpy as np
import ml_dtypes

import concourse.bacc as bacc
import concourse.mybir as mybir
import concourse.tile as tile
from concourse.bass_utils import run_bass_kernel_spmd
from concourse.masks import make_identity

P = 128
D = 1024
S = 2048
B = 4
N_CORES = 8
EO = D // P            # 8 contraction subtiles of 128
EP = EO // 2           # 4 DoubleRow pairs
DO = D // P            # 8 output-dim subtiles
KC = S // 512          # 4 k-chunks of 512
NQ = 8                 # q-slots per core
# slot j -> (n_chunks, pair_idx); global q-tile g = 4*(n-1) + 2*h + p
SLOTS = [(1, 0), (1, 1), (2, 0), (2, 1), (3, 0), (3, 1), (4, 0), (4, 1)]
F32 = mybir.dt.float32
F8 = mybir.dt.float8e4
BF16 = mybir.dt.bfloat16
NEG = -1.0e9
WQK_SCALE = 256.0      # Wq/Wk fp8 pre-scale (undone in the bias activation)
WV_SCALE = 32.0        # Wv fp8 pre-scale (undone in the denominator);
                       # kept low: device fp8e4 saturates to non-finite
                       # above ~240, and |v| can reach ~4.1
# The on-device f32->fp8 casts truncate (round toward zero).  Pre-scaling a
# value by ~half a relative ulp before the cast re-centers the quantization
# error; the factor is folded into activation scales / the denominator.
RTZ_C = 1.046875
NP_F8 = ml_dtypes.float8_e4m3fn
NP_BF16 = ml_dtypes.bfloat16

_PROG_CACHE = {}


def _slot_gtiles(h, causal):
    if causal:
        return [4 * (n - 1) + 2 * h + p for (n, p) in SLOTS]
    return [8 * h + j for j in range(NQ)]


def build_program(causal: bool):
    nc = bacc.Bacc(trn_type="TRN2", target_bir_lowering=False, debug=False)

    def din(name, shape, dt=F32):
        return nc.dram_tensor(name, shape, dt, kind="ExternalInput").ap()

    xq = din("xq", [2, P, EO, 512], F8)
    xk = din("xk", [KC, P, EO, 512], F8)
    xv = din("xv", [16, P, EO, P], F8)
    wq = din("wq", [P, EO, D], F8)
    wk = din("wk", [P, EO, D], F8)
    wv = din("wv", [P, EO, D], F8)
    bq = din("bq", [P, DO])
    bk = din("bk", [P, DO])
    bq2 = din("bq2", [P, DO])      # bq * RTZ_C, for the fp8 projection path
    bk2 = din("bk2", [P, DO])
    bv = din("bv", [P, D])
    cm = din("cm", [P, 2, 512])
    if causal:
        xq0 = din("xq0", [P, EO, P], BF16)
        xk0 = din("xk0", [P, EO, P], BF16)
        xv0 = din("xv0", [P, EO, P], BF16)
        wqb = din("wqb", [P, EO, D], BF16)
        wkb = din("wkb", [P, EO, D], BF16)
        wvb = din("wvb", [P, EO, D], BF16)
        cm0 = din("cm0", [P, P])
    out = nc.dram_tensor("out", [NQ, P, D], F32, kind="ExternalOutput").ap()
    if causal:
        outg0 = nc.dram_tensor("outg0", [P, D], F32, kind="ExternalOutput").ap()

    Ident = mybir.ActivationFunctionType.Identity
    Exp = mybir.ActivationFunctionType.Exp
    Copy = mybir.ActivationFunctionType.Copy
    add = mybir.AluOpType.add
    mult = mybir.AluOpType.mult
    DR = mybir.MatmulPerfMode.DoubleRow

    nchunks = [n for (n, _) in SLOTS] if causal else [KC] * NQ
    inv_s = float(1.0 / np.sqrt(D))
    inv_wqk2 = float(RTZ_C / WQK_SCALE)
    exp_scale = float(1.0 / (np.sqrt(D) * RTZ_C * RTZ_C))
    exp_bias = float(np.log(RTZ_C))

    with tile.TileContext(nc, pool_alloc_mode="queue") as tc, ExitStack() as top:
        const = top.enter_context(tc.tile_pool(name="const", bufs=1))
        ident8 = const.tile([P, P], F8)
        identb = const.tile([P, P], BF16)
        ident32 = const.tile([P, P], F32)
        make_identity(nc, ident32)
        nc.vector.tensor_copy(out=ident8, in_=ident32)
        nc.vector.tensor_copy(out=identb, in_=ident32)
        bq_sb = const.tile([P, DO], F32)
        nc.gpsimd.dma_start(out=bq_sb, in_=bq)
        bk_sb = const.tile([P, DO], F32)
        nc.gpsimd.dma_start(out=bk_sb, in_=bk)
        bq2_sb = const.tile([P, DO], F32)
        nc.gpsimd.dma_start(out=bq2_sb, in_=bq2)
        bk2_sb = const.tile([P, DO], F32)
        nc.gpsimd.dma_start(out=bk2_sb, in_=bk2)
        bv_sb = const.tile([P, D], F32)
        nc.gpsimd.dma_start(out=bv_sb, in_=bv)
        cm_sb = const.tile([P, 2, 512], F32)
        ebias_sb = const.tile([P, 1], F32)
        nc.gpsimd.memset(ebias_sb, exp_bias)
        if causal:
            cm0_sb = const.tile([P, P], F32)

        # persistent SBUF-resident activations
        res = top.enter_context(tc.tile_pool(name="res", bufs=1))
        qt_sb = res.tile([P, DO, 1024], F8, name="qt_sb")     # q^T
        kt_sb = res.tile([P, DO, 2048], F8, name="kt_sb")     # k^T
        v_sb = res.tile([P, 16, D], F8, name="v_sb")          # v (x WV_SCALE)
        if causal:
            qt0_sb = res.tile([P, DO, P], BF16, name="qt0_sb")
            kt0_sb = res.tile([P, DO, P], BF16, name="kt0_sb")
            v0_sb = res.tile([P, D], BF16, name="v0_sb")
        accp = top.enter_context(tc.tile_pool(name="accp", bufs=1))
        accs = [accp.tile([P, KC], F32, name=f"acc{j}") for j in range(NQ)]
        if causal:
            accg = accp.tile([P, 1], F32, name="accg")

        # ---------------- projections ----------------
        # All input-DMA triggers are emitted up front so the two hardware
        # DGE rings (sync / scalar) start streaming at t=0 and no trigger
        # gets stuck behind activations in the scalar engine's queue.
        # Every input tile is a distinct pool slot (no WAR deps on triggers).
        with nc.named_scope("proj"), \
             tc.tile_pool(name="wt8", bufs=3) as wt8, \
             tc.tile_pool(name="wtb", bufs=3) as wtb, \
             tc.tile_pool(name="xin", bufs=6) as xinp, \
             tc.tile_pool(name="xvp", bufs=16) as xvp, \
             tc.tile_pool(name="xg0", bufs=3) as xg0, \
             tc.tile_pool(name="psA", bufs=4, space="PSUM") as psA, \
             tc.tile_pool(name="psB", bufs=2, space="PSUM") as psB:

            wk_sb = wt8.tile([P, EO, D], F8, tag="wt8", name="wk_sb")
            wq_sb = wt8.tile([P, EO, D], F8, tag="wt8", name="wq_sb")
            wv_sb = wt8.tile([P, EO, D], F8, tag="wt8", name="wv_sb")
            xk_ts = [xinp.tile([P, EO, 512], F8, tag="xin", name=f"xk_t{kc}")
                     for kc in range(KC)]
            xq_ts = [xinp.tile([P, EO, 512], F8, tag="xin", name=f"xq_t{sc}")
                     for sc in range(2)]
            xv_ts = [xvp.tile([P, EO, P], F8, tag="xv", name=f"xv_t{st}")
                     for st in range(16)]
            if causal:
                wqb_sb = wtb.tile([P, EO, D], BF16, tag="wtb", name="wqb_sb")
                wkb_sb = wtb.tile([P, EO, D], BF16, tag="wtb", name="wkb_sb")
                wvb_sb = wtb.tile([P, EO, D], BF16, tag="wtb", name="wvb_sb")
                xq0_t = xg0.tile([P, EO, P], BF16, tag="xg0", name="xq0_t")
                xk0_t = xg0.tile([P, EO, P], BF16, tag="xg0", name="xk0_t")
                xv0_t = xg0.tile([P, EO, P], BF16, tag="xg0", name="xv0_t")

            # DMA trigger instructions cost ~0.6-0.8us on the issuing engine
            # and BLOCK it when the hardware ring backs up.  The scalar
            # engine (which must run the projection activations from ~12us)
            # therefore issues only the 8 earliest triggers; the sync and
            # gpsimd engines, which have no compute, carry everything else.
            # scalar ring: K-chunk-0 + Q-slab-0 inputs only.  Whole-tile
            # transfers: contiguous 4-8KB partition lines run ~4x faster
            # than the 1-2KB strided per-pair slices.
            nc.scalar.dma_start(out=xk_ts[0], in_=xk[0])
            nc.scalar.dma_start(out=xq_ts[0], in_=xq[0])
            # sync ring: wk first (K0 critical path), then inputs in
            # consumption order, bf16 g0 weights last
            nc.sync.dma_start(out=wk_sb, in_=wk)
            nc.sync.dma_start(out=xq_ts[1], in_=xq[1])
            nc.sync.dma_start(out=wv_sb, in_=wv)
            nc.sync.dma_start(out=xk_ts[1], in_=xk[1])
            nc.sync.dma_start(out=xk_ts[2], in_=xk[2])
            nc.sync.dma_start(out=xk_ts[3], in_=xk[3])
            if causal:
                nc.sync.dma_start(out=wqb_sb, in_=wqb)
            # gpsimd (software DGE, ~135 GB/s measured): wq early, then v
            # tiles, g0 inputs, and the two late bf16 weights
            nc.gpsimd.dma_start(out=wq_sb, in_=wq)
            for st in range(16):
                nc.gpsimd.dma_start(out=xv_ts[st], in_=xv[st])
            nc.gpsimd.dma_start(out=cm_sb, in_=cm)
            if causal:
                nc.gpsimd.dma_start(out=cm0_sb, in_=cm0)
                nc.gpsimd.dma_start(out=xq0_t, in_=xq0)
                nc.gpsimd.dma_start(out=xk0_t, in_=xk0)
                nc.gpsimd.dma_start(out=xv0_t, in_=xv0)
                nc.gpsimd.dma_start(out=wkb_sb, in_=wkb)
                nc.gpsimd.dma_start(out=wvb_sb, in_=wvb)

            def kq_chunk(w_sb, x_t, kt_dst, b_sb):
                """one 512-col slab of the K or Q projection (fp8 DoubleRow).

                The psum->fp8 drain alternates scalar/vector so psum slot
                recycling is not bound by a single engine's latency."""
                for do in range(DO):
                    ps = psA.tile([P, 512], F32, tag="psA", name="pskq")
                    for ep in range(EP):
                        nc.tensor.matmul(
                            ps,
                            lhsT=w_sb[:, 2 * ep:2 * ep + 2, do * P:(do + 1) * P],
                            rhs=x_t[:, 2 * ep:2 * ep + 2, :],
                            start=(ep == 0), stop=(ep == EP - 1),
                            perf_mode=DR)
                    if do % 2 == 0:
                        nc.scalar.activation(out=kt_dst[:, do, :], in_=ps,
                                             func=Ident, scale=inv_wqk2,
                                             bias=b_sb[:, do:do + 1])
                    else:
                        nc.vector.tensor_scalar(
                            out=kt_dst[:, do, :], in0=ps,
                            scalar1=inv_wqk2, scalar2=b_sb[:, do:do + 1],
                            op0=mult, op1=add)

            def v_tile(st):
                xv_t = xv_ts[st]
                ps2 = psB.tile([P, D], F32, tag="psB", name=f"psv{st}")
                for half in range(2):
                    for ep in range(EP):
                        nc.tensor.matmul(
                            ps2[:, half * 512:(half + 1) * 512],
                            lhsT=xv_t[:, 2 * ep:2 * ep + 2, :],
                            rhs=wv_sb[:, 2 * ep:2 * ep + 2,
                                      half * 512:(half + 1) * 512],
                            start=(ep == 0), stop=(ep == EP - 1),
                            perf_mode=DR)
                if st % 2 == 0:
                    nc.vector.tensor_scalar_mul(v_sb[:, st, :], ps2, RTZ_C)
                else:
                    nc.scalar.activation(out=v_sb[:, st, :], in_=ps2,
                                         func=Copy, scale=RTZ_C)

            def kq_chunk_eo(w_sb, x_t, kt_dst, b_sb):
                """ep-outer variant for the startup phases: weight pairs are
                consumed incrementally as their DMAs land, instead of every
                do-block stalling on the full weight tile."""
                for half in range(2):
                    dos = range(half * 4, half * 4 + 4)
                    pss = [psA.tile([P, 512], F32, tag="psA",
                                    name=f"pseo{half}_{i}")
                           for i in range(4)]
                    for ep in range(EP):
                        for i, do in enumerate(dos):
                            nc.tensor.matmul(
                                pss[i],
                                lhsT=w_sb[:, 2 * ep:2 * ep + 2,
                                          do * P:(do + 1) * P],
                                rhs=x_t[:, 2 * ep:2 * ep + 2, :],
                                start=(ep == 0), stop=(ep == EP - 1),
                                perf_mode=DR)
                    for i, do in enumerate(dos):
                        if do % 2 == 0:
                            nc.scalar.activation(out=kt_dst[:, do, :],
                                                 in_=pss[i], func=Ident,
                                                 scale=inv_wqk2,
                                                 bias=b_sb[:, do:do + 1])
                        else:
                            nc.vector.tensor_scalar(
                                out=kt_dst[:, do, :], in0=pss[i],
                                scalar1=inv_wqk2, scalar2=b_sb[:, do:do + 1],
                                op0=mult, op1=add)

            # K chunk 0 first (earliest attention dependency)
            kq_chunk_eo(wk_sb, xk_ts[0], kt_sb[:, :, 0:512], bk2_sb)

            # Q projection (both slabs)
            kq_chunk_eo(wq_sb, xq_ts[0], qt_sb[:, :, 0:512], bq2_sb)
            kq_chunk(wq_sb, xq_ts[1], qt_sb[:, :, 512:1024], bq2_sb)

            # V tiles interleaved with the remaining K chunks; the g0 bf16
            # side path runs last, by which time its weights (low-priority
            # tail of both hardware rings) have arrived.
            for st in range(4):
                v_tile(st)
            kq_chunk(wk_sb, xk_ts[1], kt_sb[:, :, 512:1024], bk2_sb)
            for st in range(4, 8):
                v_tile(st)
            for kc in range(2, KC):
                kq_chunk(wk_sb, xk_ts[kc],
                         kt_sb[:, :, kc * 512:(kc + 1) * 512], bk2_sb)
                for st in range(4 * kc, 4 * kc + 4):
                    v_tile(st)

            if causal:
                for (wb, xt, dst, bb) in ((wqb_sb, xq0_t, qt0_sb, bq_sb),
                                          (wkb_sb, xk0_t, kt0_sb, bk_sb)):
                    for do in range(DO):
                        psg = psA.tile([P, P], F32, tag="psA", name="psg")
                        for eo in range(EO):
                            nc.tensor.matmul(
                                psg,
                                lhsT=wb[:, eo, do * P:(do + 1) * P],
                                rhs=xt[:, eo, :],
                                start=(eo == 0), stop=(eo == EO - 1))
                        nc.scalar.activation(out=dst[:, do, :], in_=psg,
                                             func=Ident,
                                             bias=bb[:, do:do + 1])
                psv0 = psB.tile([P, D], F32, tag="psB", name="psv0")
                for half in range(2):
                    for eo in range(EO):
                        nc.tensor.matmul(
                            psv0[:, half * 512:(half + 1) * 512],
                            lhsT=xv0_t[:, eo, :],
                            rhs=wvb_sb[:, eo, half * 512:(half + 1) * 512],
                            start=(eo == 0), stop=(eo == EO - 1))
                nc.vector.tensor_copy(out=v0_sb, in_=psv0)

        # ---------------- attention, q-major, software-pipelined ----------
        with nc.named_scope("attn"), \
             tc.tile_pool(name="pep", bufs=3) as pep, \
             tc.tile_pool(name="ptp", bufs=3) as ptp, \
             tc.tile_pool(name="denp", bufs=4) as denp, \
             tc.tile_pool(name="outp", bufs=3) as outp, \
             tc.tile_pool(name="psS", bufs=2, space="PSUM") as psS, \
             tc.tile_pool(name="psT", bufs=2, space="PSUM") as psT, \
             tc.tile_pool(name="psV", bufs=2, space="PSUM") as psV:

            items = [(j, c) for j in range(NQ) for c in range(nchunks[j])]
            units = (["g0"] if causal else []) + items
            state = {}
            av_of = {}

            def emit_S(u):
                if u == "g0":
                    psg = psS.tile([P, P], F32, tag="s", name="psg0")
                    for do in range(DO):
                        nc.tensor.matmul(
                            psg, lhsT=qt0_sb[:, do, :], rhs=kt0_sb[:, do, :],
                            start=(do == 0), stop=(do == DO - 1))
                    nc.vector.tensor_tensor(out=psg, in0=psg, in1=cm0_sb, op=add)
                    peg = pep.tile([P, P], BF16, tag="p", name="peg0")
                    nc.scalar.activation(out=peg, in_=psg, func=Exp,
                                         scale=inv_s, accum_out=accg)
                    state[u] = peg
                    return
                j, c = u
                ps = psS.tile([P, 512], F32, tag="s", name=f"ps{c}_{j}")
                for ep in range(EP):
                    nc.tensor.matmul(
                        ps,
                        lhsT=qt_sb[:, 2 * ep:2 * ep + 2, j * P:(j + 1) * P],
                        rhs=kt_sb[:, 2 * ep:2 * ep + 2, c * 512:(c + 1) * 512],
                        start=(ep == 0), stop=(ep == EP - 1),
                        perf_mode=DR)
                if causal and c == nchunks[j] - 1:
                    p_j = SLOTS[j][1]
                    nc.vector.tensor_tensor(out=ps, in0=ps,
                                            in1=cm_sb[:, p_j, :], op=add)
                pe = pep.tile([P, 512], BF16, tag="p", name=f"pe{c}_{j}")
                nc.scalar.activation(out=pe, in_=ps, func=Exp,
                                     scale=exp_scale, bias=ebias_sb,
                                     accum_out=accs[j][:, c:c + 1])
                state[u] = pe

            def emit_T(u):
                pe = state[u]
                if u == "g0":
                    ptps = psT.tile([P, P], BF16, tag="t", name="ptg0")
                    nc.tensor.transpose(ptps, pe, identb)
                    pt = ptp.tile([P, P], BF16, tag="pt", name="ptg0_sb")
                    nc.vector.tensor_copy(out=pt, in_=ptps)
                else:
                    j, c = u
                    ptps = psT.tile([P, 4, P], BF16, tag="t", name=f"ptps{c}_{j}")
                    for t in range(4):
                        nc.tensor.transpose(ptps[:, t, :],
                                            pe[:, t * P:(t + 1) * P], identb)
                    pt = ptp.tile([P, 4, P], F8, tag="pt", name=f"pt{c}_{j}")
                    nc.vector.tensor_copy(out=pt, in_=ptps)
                state[u] = pt

            def emit_A(u):
                pt = state.pop(u)
                if u == "g0":
                    avg = psV.tile([P, D], F32, tag="avp", name="avg0")
                    for half in range(2):
                        nc.tensor.matmul(
                            avg[:, half * 512:(half + 1) * 512],
                            lhsT=pt,
                            rhs=v0_sb[:, half * 512:(half + 1) * 512],
                            start=True, stop=True)
                    recg = denp.tile([P, 1], F32, tag="rec", name="recg0")
                    nc.vector.reciprocal(out=recg, in_=accg)
                    og = outp.tile([P, D], F32, tag="o", name="og0")
                    nc.vector.scalar_tensor_tensor(
                        out=og, in0=avg, scalar=recg, in1=bv_sb,
                        op0=mult, op1=add)
                    nc.sync.dma_start(out=outg0, in_=og)
                    return
                j, c = u
                n_j = nchunks[j]
                if c == 0:
                    av_of[j] = psV.tile([P, D], F32, tag="avp", name=f"av{j}")
                av = av_of[j]
                for tp in range(2):
                    for half in range(2):
                        nc.tensor.matmul(
                            av[:, half * 512:(half + 1) * 512],
                            lhsT=pt[:, 2 * tp:2 * tp + 2, :],
                            rhs=v_sb[:, 4 * c + 2 * tp:4 * c + 2 * tp + 2,
                                     half * 512:(half + 1) * 512],
                            start=(c == 0 and tp == 0),
                            stop=(c == n_j - 1 and tp == 1),
                            perf_mode=DR)
                if c == n_j - 1:
                    den = denp.tile([P, 1], F32, tag="den", name=f"den{j}")
                    nc.vector.tensor_reduce(
                        out=den, in_=accs[j][:, 0:n_j],
                        axis=mybir.AxisListType.X, op=add)
                    den64 = denp.tile([P, 1], F32, tag="den64", name=f"den64{j}")
                    nc.vector.tensor_scalar_mul(den64, den, float(WV_SCALE * RTZ_C))
                    rec = denp.tile([P, 1], F32, tag="rec", name=f"rec{j}")
                    nc.vector.reciprocal(out=rec, in_=den64)
                    o = outp.tile([P, D], F32, tag="o", name=f"o{j}")
                    nc.vector.scalar_tensor_tensor(
                        out=o, in0=av_of.pop(j), scalar=rec, in1=bv_sb,
                        op0=mult, op1=add)
                    nc.sync.dma_start(out=out[j], in_=o)

            N = len(units)
            for t in range(N + 2):
                if t < N:
                    emit_S(units[t])
                if 1 <= t <= N:
                    emit_T(units[t - 1])
                if t >= 2:
                    emit_A(units[t - 2])

    nc.compile()
    return nc


def _get_program(causal: bool):
    key = bool(causal)
    if key not in _PROG_CACHE:
        _PROG_CACHE[key] = build_program(key)
    return _PROG_CACHE[key]


def _shard_inputs(encoded_q, encoded_k, encoded_v, W_q, b_q, W_k, b_k,
                  W_v, b_v, causal):
    """Build the per-core in_maps (all host-side numpy)."""
    def wlayout(W, scale, dt):
        return np.ascontiguousarray(
            (W.T * scale).reshape(EO, P, D).transpose(1, 0, 2)).astype(dt)

    wq8 = wlayout(W_q, WQK_SCALE, NP_F8)
    wk8 = wlayout(W_k, WQK_SCALE, NP_F8)
    wv8 = wlayout(W_v, WV_SCALE, NP_F8)
    bqh = np.ascontiguousarray(b_q.reshape(DO, P).T)
    bkh = np.ascontiguousarray(b_k.reshape(DO, P).T)
    bvh = np.ascontiguousarray(np.broadcast_to(b_v, (P, D)))
    if causal:
        wqb = wlayout(W_q, 1.0, NP_BF16)
        wkb = wlayout(W_k, 1.0, NP_BF16)
        wvb = wlayout(W_v, 1.0, NP_BF16)
        qi = np.arange(P)[:, None]
        cm0h = np.where(np.arange(P)[None, :] <= qi, 0.0, NEG).astype(np.float32)

    kf = np.arange(512)[None, :]
    in_maps = []
    for c in range(N_CORES):
        b, h = divmod(c, 2)
        gts = _slot_gtiles(h, causal)
        Xq = np.concatenate([encoded_q[b, g * P:(g + 1) * P, :] for g in gts], 0)
        xqh = np.ascontiguousarray(
            Xq.T.reshape(EO, P, 2, 512).transpose(2, 1, 0, 3)).astype(NP_F8)
        xkh = np.ascontiguousarray(
            encoded_k[b].T.reshape(EO, P, KC, 512).transpose(2, 1, 0, 3)
        ).astype(NP_F8)
        xvh = np.ascontiguousarray(
            encoded_v[b].T.reshape(EO, P, 16, P).transpose(2, 1, 0, 3)
        ).astype(NP_F8)
        cmh = np.zeros((P, 2, 512), np.float32)
        im = {
            "xq": xqh, "xk": xkh, "xv": xvh,
            "wq": wq8, "wk": wk8, "wv": wv8,
            "bq": bqh, "bk": bkh, "bv": bvh, "cm": cmh,
            "bq2": np.float32(RTZ_C) * bqh, "bk2": np.float32(RTZ_C) * bkh,
        }
        if causal:
            qi = np.arange(P)[:, None]
            for p in range(2):
                r = 2 * h + p
                cmh[:, p, :] = np.where(kf <= r * P + qi, 0.0, NEG)
            def x0layout(X):
                return np.ascontiguousarray(
                    X[b, :P, :].T.reshape(EO, P, P).transpose(1, 0, 2)
                ).astype(NP_BF16)
            im.update({
                "xq0": x0layout(encoded_q), "xk0": x0layout(encoded_k),
                "xv0": x0layout(encoded_v),
                "wqb": wqb, "wkb": wkb, "wvb": wvb, "cm0": cm0h,
            })
        in_maps.append(im)
    return in_maps


def kernel(encoded_q, encoded_k, encoded_v, W_q, b_q, W_k, b_k, W_v, b_v,
           parameter_mask, _want_trace=False, _trace_dir=None):
    causal = bool(np.asarray(parameter_mask).item())
    encoded_q = np.asarray(encoded_q, np.float32)
    encoded_k = np.asarray(encoded_k, np.float32)
    encoded_v = np.asarray(encoded_v, np.float32)
    nc = _get_program(causal)
    in_maps = _shard_inputs(encoded_q, encoded_k, encoded_v,
                            np.asarray(W_q, np.float32), np.asarray(b_q, np.float32),
                            np.asarray(W_k, np.float32), np.asarray(b_k, np.float32),
                            np.asarray(W_v, np.float32), np.asarray(b_v, np.float32),
                            causal)
    kw = {}
    if _want_trace:
        kw = dict(trace=True, tmpdir=_trace_dir)
    res = run_bass_kernel_spmd(nc, in_maps, core_ids=list(range(N_CORES)), **kw)

    full = np.empty((B, S, D), np.float32)
    for c in range(N_CORES):
        b, h = divmod(c, 2)
        o = res.results[c]["out"]
        for j, g in enumerate(_slot_gtiles(h, causal)):
            full[b, g * P:(g + 1) * P, :] = o[j]
        if causal and h == 0:
            full[b, 0:P, :] = res.results[c]["outg0"]
    if _want_trace:
        return full, res
    return full


# revision 63
# speedup vs baseline: 1.1009x; 1.0458x over previous
"""Trainium2 Bass kernel for single-head causal attention (fp8 version).

  q = Xq @ Wq.T + bq ; k = Xk @ Wk.T + bk ; v = Xv @ Wv.T + bv
  out = softmax((q k^T + causal_mask)/sqrt(D)) @ v

Shapes: B=4, S=2048, D=1024, fp32 in/out.  8 NeuronCores, SPMD.

Sharding (uniform causal schedule -> identical program on every core):
  core c handles batch b = c//2, parity h = c%2.
  Within the batch, S splits into 16 q-tiles of 128.  q-tile g needs
  ceil((g+1)/4) k-chunks of 512.  Core (b, h) takes, for every chunk-count
  class n in {1,2,3,4}, the two tiles g = 4*(n-1) + 2*h and +2*h+1.  Each
  core owns 8 q-tiles with chunk counts [1,1,2,2,3,3,4,4].

Precision plan (rel-err budget 2e-2; emulated max err ~7e-3):
  - Q/K/V projections run in fp8e4 (e4m3) with DoubleRow perf mode
    (0.5 PE cycles/row): weights pre-scaled x256 (Wq/Wk; /256 folded into
    the bias activation) or x64 (Wv; /64 folded into the softmax
    denominator reciprocal).
  - scores (q k^T) and attn@V also fp8 DoubleRow; P transposed on the PE.
  - K^T / V / Q^T all stay SBUF-resident (fp8: 16+16+8 KB/partition).
  - few-key softmax rows (0..127) are precision-critical: a small bf16
    side path recomputes q/k/v tiles for rows/keys<128 and produces the
    g-tile-0 output (used by h=0 cores; ignored by h=1).
"""

from contextlib import ExitStack

import numpy as np
import ml_dtypes

import concourse.bacc as bacc
import concourse.mybir as mybir
import concourse.tile as tile
from concourse.bass_utils import run_bass_kernel_spmd
from concourse.masks import make_identity

P = 128
D = 1024
S = 2048
B = 4
N_CORES = 8
EO = D // P            # 8 contraction subtiles of 128
EP = EO // 2           # 4 DoubleRow pairs
DO = D // P            # 8 output-dim subtiles
KC = S // 512          # 4 k-chunks of 512
NQ = 8                 # q-slots per core
# slot j -> (n_chunks, pair_idx); global q-tile g = 4*(n-1) + 2*h + p
SLOTS = [(1, 0), (1, 1), (2, 0), (2, 1), (3, 0), (3, 1), (4, 0), (4, 1)]
F32 = mybir.dt.float32
F8 = mybir.dt.float8e4
BF16 = mybir.dt.bfloat16
NEG = -1.0e9
WQK_SCALE = 256.0      # Wq/Wk fp8 pre-scale (undone in the bias activation)
WV_SCALE = 32.0        # Wv fp8 pre-scale (undone in the denominator);
                       # kept low: device fp8e4 saturates to non-finite
                       # above ~240, and |v| can reach ~4.1
# The on-device f32->fp8 casts truncate (round toward zero).  Pre-scaling a
# value by ~half a relative ulp before the cast re-centers the quantization
# error; the factor is folded into activation scales / the denominator.
RTZ_C = 1.046875
NP_F8 = ml_dtypes.float8_e4m3fn
NP_BF16 = ml_dtypes.bfloat16

_PROG_CACHE = {}


def _slot_gtiles(h, causal):
    if causal:
        return [4 * (n - 1) + 2 * h + p for (n, p) in SLOTS]
    return [8 * h + j for j in range(NQ)]


def build_program(causal: bool):
    nc = bacc.Bacc(trn_type="TRN2", target_bir_lowering=False, debug=False)

    def din(name, shape, dt=F32):
        return nc.dram_tensor(name, shape, dt, kind="ExternalInput").ap()

    xq = din("xq", [2, P, EO, 512], F8)
    xk = din("xk", [KC, P, EO, 512], F8)
    xv = din("xv", [16, P, EO, P], F8)
    wq = din("wq", [P, EO, D], F8)
    wk = din("wk", [P, EO, D], F8)
    wv = din("wv", [P, EO, D], F8)
    bq = din("bq", [P, DO])
    bk = din("bk", [P, DO])
    bq2 = din("bq2", [P, DO])      # bq * RTZ_C, for the fp8 projection path
    bk2 = din("bk2", [P, DO])
    bv = din("bv", [P, D])
    cm = din("cm", [P, 2, 512])
    if causal:
        xq0 = din("xq0", [P, EO, P], BF16)
        xk0 = din("xk0", [P, EO, P], BF16)
        xv0 = din("xv0", [P, EO, P], BF16)
        wqb = din("wqb", [P, EO, D], BF16)
        wkb = din("wkb", [P, EO, D], BF16)
        wvb = din("wvb", [P, EO, D], BF16)
        cm0 = din("cm0", [P, P])
    out = nc.dram_tensor("out", [NQ, P, D], F32, kind="ExternalOutput").ap()
    if causal:
        outg0 = nc.dram_tensor("outg0", [P, D], F32, kind="ExternalOutput").ap()

    Ident = mybir.ActivationFunctionType.Identity
    Exp = mybir.ActivationFunctionType.Exp
    Copy = mybir.ActivationFunctionType.Copy
    add = mybir.AluOpType.add
    mult = mybir.AluOpType.mult
    DR = mybir.MatmulPerfMode.DoubleRow

    nchunks = [n for (n, _) in SLOTS] if causal else [KC] * NQ
    inv_s = float(1.0 / np.sqrt(D))
    inv_wqk2 = float(RTZ_C / WQK_SCALE)
    exp_scale = float(1.0 / (np.sqrt(D) * RTZ_C * RTZ_C))
    exp_bias = float(np.log(RTZ_C))

    with tile.TileContext(nc, pool_alloc_mode="queue") as tc, ExitStack() as top:
        const = top.enter_context(tc.tile_pool(name="const", bufs=1))
        ident8 = const.tile([P, P], F8)
        identb = const.tile([P, P], BF16)
        ident32 = const.tile([P, P], F32)
        make_identity(nc, ident32)
        nc.vector.tensor_copy(out=ident8, in_=ident32)
        nc.vector.tensor_copy(out=identb, in_=ident32)
        bq_sb = const.tile([P, DO], F32)
        nc.gpsimd.dma_start(out=bq_sb, in_=bq)
        bk_sb = const.tile([P, DO], F32)
        nc.gpsimd.dma_start(out=bk_sb, in_=bk)
        bq2_sb = const.tile([P, DO], F32)
        nc.gpsimd.dma_start(out=bq2_sb, in_=bq2)
        bk2_sb = const.tile([P, DO], F32)
        nc.gpsimd.dma_start(out=bk2_sb, in_=bk2)
        bv_sb = const.tile([P, D], F32)
        nc.gpsimd.dma_start(out=bv_sb, in_=bv)
        cm_sb = const.tile([P, 2, 512], F32)
        ebias_sb = const.tile([P, 1], F32)
        nc.gpsimd.memset(ebias_sb, exp_bias)
        if causal:
            cm0_sb = const.tile([P, P], F32)

        # persistent SBUF-resident activations
        res = top.enter_context(tc.tile_pool(name="res", bufs=1))
        qt_sb = res.tile([P, DO, 1024], F8, name="qt_sb")     # q^T
        kt_sb = res.tile([P, DO, 2048], F8, name="kt_sb")     # k^T
        v_sb = res.tile([P, 16, D], F8, name="v_sb")          # v (x WV_SCALE)
        if causal:
            qt0_sb = res.tile([P, DO, P], BF16, name="qt0_sb")
            kt0_sb = res.tile([P, DO, P], BF16, name="kt0_sb")
            v0_sb = res.tile([P, D], BF16, name="v0_sb")
        accp = top.enter_context(tc.tile_pool(name="accp", bufs=1))
        accs = [accp.tile([P, KC], F32, name=f"acc{j}") for j in range(NQ)]
        if causal:
            accg = accp.tile([P, 1], F32, name="accg")

        # ---------------- projections ----------------
        # All input-DMA triggers are emitted up front so the two hardware
        # DGE rings (sync / scalar) start streaming at t=0 and no trigger
        # gets stuck behind activations in the scalar engine's queue.
        # Every input tile is a distinct pool slot (no WAR deps on triggers).
        with nc.named_scope("proj"), \
             tc.tile_pool(name="wt8", bufs=3) as wt8, \
             tc.tile_pool(name="wtb", bufs=3) as wtb, \
             tc.tile_pool(name="xin", bufs=6) as xinp, \
             tc.tile_pool(name="xvp", bufs=16) as xvp, \
             tc.tile_pool(name="xg0", bufs=3) as xg0, \
             tc.tile_pool(name="psA", bufs=4, space="PSUM") as psA, \
             tc.tile_pool(name="psB", bufs=2, space="PSUM") as psB:

            wk_sb = wt8.tile([P, EO, D], F8, tag="wt8", name="wk_sb")
            wq_sb = wt8.tile([P, EO, D], F8, tag="wt8", name="wq_sb")
            wv_sb = wt8.tile([P, EO, D], F8, tag="wt8", name="wv_sb")
            xk_ts = [xinp.tile([P, EO, 512], F8, tag="xin", name=f"xk_t{kc}")
                     for kc in range(KC)]
            xq_ts = [xinp.tile([P, EO, 512], F8, tag="xin", name=f"xq_t{sc}")
                     for sc in range(2)]
            xv_ts = [xvp.tile([P, EO, P], F8, tag="xv", name=f"xv_t{st}")
                     for st in range(16)]
            if causal:
                wqb_sb = wtb.tile([P, EO, D], BF16, tag="wtb", name="wqb_sb")
                wkb_sb = wtb.tile([P, EO, D], BF16, tag="wtb", name="wkb_sb")
                wvb_sb = wtb.tile([P, EO, D], BF16, tag="wtb", name="wvb_sb")
                xq0_t = xg0.tile([P, EO, P], BF16, tag="xg0", name="xq0_t")
                xk0_t = xg0.tile([P, EO, P], BF16, tag="xg0", name="xk0_t")
                xv0_t = xg0.tile([P, EO, P], BF16, tag="xg0", name="xv0_t")

            # DMA trigger instructions cost ~0.6-0.8us on the issuing engine
            # and BLOCK it when the hardware ring backs up.  The scalar
            # engine (which must run the projection activations from ~12us)
            # therefore issues only the 8 earliest triggers; the sync and
            # gpsimd engines, which have no compute, carry everything else.
            # scalar ring: K-chunk-0 + Q-slab-0 inputs only.  Whole-tile
            # transfers: contiguous 4-8KB partition lines run ~4x faster
            # than the 1-2KB strided per-pair slices.
            nc.scalar.dma_start(out=xk_ts[0][:, 0:4, :], in_=xk[0, :, 0:4, :])
            nc.scalar.dma_start(out=xk_ts[0][:, 4:8, :], in_=xk[0, :, 4:8, :])
            nc.scalar.dma_start(out=xq_ts[0], in_=xq[0])
            # sync ring: wk first (K0 critical path, in halves so the
            # ep-outer K0 loop starts on the first half), wq next, then
            # inputs in consumption order, bf16 g0 weights last
            nc.sync.dma_start(out=wk_sb[:, 0:4, :], in_=wk[:, 0:4, :])
            nc.sync.dma_start(out=wk_sb[:, 4:8, :], in_=wk[:, 4:8, :])
            nc.sync.dma_start(out=wq_sb, in_=wq)
            nc.sync.dma_start(out=xq_ts[1], in_=xq[1])
            nc.sync.dma_start(out=wv_sb, in_=wv)
            nc.sync.dma_start(out=xk_ts[1], in_=xk[1])
            nc.sync.dma_start(out=xk_ts[2], in_=xk[2])
            nc.sync.dma_start(out=xk_ts[3], in_=xk[3])
            if causal:
                nc.sync.dma_start(out=wqb_sb, in_=wqb)
            # gpsimd (software DGE, ~135 GB/s measured): v tiles, g0
            # inputs, and the two late bf16 weights
            for st in range(16):
                nc.gpsimd.dma_start(out=xv_ts[st], in_=xv[st])
            nc.gpsimd.dma_start(out=cm_sb, in_=cm)
            if causal:
                nc.gpsimd.dma_start(out=cm0_sb, in_=cm0)
                nc.gpsimd.dma_start(out=xq0_t, in_=xq0)
                nc.gpsimd.dma_start(out=xk0_t, in_=xk0)
                nc.gpsimd.dma_start(out=xv0_t, in_=xv0)
                nc.gpsimd.dma_start(out=wkb_sb, in_=wkb)
                nc.gpsimd.dma_start(out=wvb_sb, in_=wvb)

            def kq_chunk(w_sb, x_t, kt_dst, b_sb):
                """one 512-col slab of the K or Q projection (fp8 DoubleRow).

                The psum->fp8 drain alternates scalar/vector so psum slot
                recycling is not bound by a single engine's latency."""
                for do in range(DO):
                    ps = psA.tile([P, 512], F32, tag="psA", name="pskq")
                    for ep in range(EP):
                        nc.tensor.matmul(
                            ps,
                            lhsT=w_sb[:, 2 * ep:2 * ep + 2, do * P:(do + 1) * P],
                            rhs=x_t[:, 2 * ep:2 * ep + 2, :],
                            start=(ep == 0), stop=(ep == EP - 1),
                            perf_mode=DR)
                    if do % 2 == 0:
                        nc.scalar.activation(out=kt_dst[:, do, :], in_=ps,
                                             func=Ident, scale=inv_wqk2,
                                             bias=b_sb[:, do:do + 1])
                    else:
                        nc.vector.tensor_scalar(
                            out=kt_dst[:, do, :], in0=ps,
                            scalar1=inv_wqk2, scalar2=b_sb[:, do:do + 1],
                            op0=mult, op1=add)

            def v_tile(st):
                xv_t = xv_ts[st]
                ps2 = psB.tile([P, D], F32, tag="psB", name=f"psv{st}")
                for half in range(2):
                    for ep in range(EP):
                        nc.tensor.matmul(
                            ps2[:, half * 512:(half + 1) * 512],
                            lhsT=xv_t[:, 2 * ep:2 * ep + 2, :],
                            rhs=wv_sb[:, 2 * ep:2 * ep + 2,
                                      half * 512:(half + 1) * 512],
                            start=(ep == 0), stop=(ep == EP - 1),
                            perf_mode=DR)
                if st % 2 == 0:
                    nc.vector.tensor_scalar_mul(v_sb[:, st, :], ps2, RTZ_C)
                else:
                    nc.scalar.activation(out=v_sb[:, st, :], in_=ps2,
                                         func=Copy, scale=RTZ_C)

            def kq_chunk_eo(w_sb, x_t, kt_dst, b_sb):
                """ep-outer variant for the startup phases: weight pairs are
                consumed incrementally as their DMAs land, instead of every
                do-block stalling on the full weight tile."""
                for half in range(2):
                    dos = range(half * 4, half * 4 + 4)
                    pss = [psA.tile([P, 512], F32, tag="psA",
                                    name=f"pseo{half}_{i}")
                           for i in range(4)]
                    for ep in range(EP):
                        for i, do in enumerate(dos):
                            nc.tensor.matmul(
                                pss[i],
                                lhsT=w_sb[:, 2 * ep:2 * ep + 2,
                                          do * P:(do + 1) * P],
                                rhs=x_t[:, 2 * ep:2 * ep + 2, :],
                                start=(ep == 0), stop=(ep == EP - 1),
                                perf_mode=DR)
                    for i, do in enumerate(dos):
                        if do % 2 == 0:
                            nc.scalar.activation(out=kt_dst[:, do, :],
                                                 in_=pss[i], func=Ident,
                                                 scale=inv_wqk2,
                                                 bias=b_sb[:, do:do + 1])
                        else:
                            nc.vector.tensor_scalar(
                                out=kt_dst[:, do, :], in0=pss[i],
                                scalar1=inv_wqk2, scalar2=b_sb[:, do:do + 1],
                                op0=mult, op1=add)

            # K chunk 0 first (earliest attention dependency)
            kq_chunk_eo(wk_sb, xk_ts[0], kt_sb[:, :, 0:512], bk2_sb)

            # Q projection (both slabs)
            kq_chunk_eo(wq_sb, xq_ts[0], qt_sb[:, :, 0:512], bq2_sb)
            kq_chunk(wq_sb, xq_ts[1], qt_sb[:, :, 512:1024], bq2_sb)

            # V tiles interleaved with the remaining K chunks; the g0 bf16
            # side path runs last, by which time its weights (low-priority
            # tail of both hardware rings) have arrived.
            for st in range(4):
                v_tile(st)
            kq_chunk(wk_sb, xk_ts[1], kt_sb[:, :, 512:1024], bk2_sb)
            for st in range(4, 8):
                v_tile(st)
            for kc in range(2, KC):
                kq_chunk(wk_sb, xk_ts[kc],
                         kt_sb[:, :, kc * 512:(kc + 1) * 512], bk2_sb)
                for st in range(4 * kc, 4 * kc + 4):
                    v_tile(st)

            if causal:
                for (wb, xt, dst, bb) in ((wqb_sb, xq0_t, qt0_sb, bq_sb),
                                          (wkb_sb, xk0_t, kt0_sb, bk_sb)):
                    for do in range(DO):
                        psg = psA.tile([P, P], F32, tag="psA", name="psg")
                        for eo in range(EO):
                            nc.tensor.matmul(
                                psg,
                                lhsT=wb[:, eo, do * P:(do + 1) * P],
                                rhs=xt[:, eo, :],
                                start=(eo == 0), stop=(eo == EO - 1))
                        nc.scalar.activation(out=dst[:, do, :], in_=psg,
                                             func=Ident,
                                             bias=bb[:, do:do + 1])
                psv0 = psB.tile([P, D], F32, tag="psB", name="psv0")
                for half in range(2):
                    for eo in range(EO):
                        nc.tensor.matmul(
                            psv0[:, half * 512:(half + 1) * 512],
                            lhsT=xv0_t[:, eo, :],
                            rhs=wvb_sb[:, eo, half * 512:(half + 1) * 512],
                            start=(eo == 0), stop=(eo == EO - 1))
                nc.vector.tensor_copy(out=v0_sb, in_=psv0)

        # ---------------- attention, q-major, software-pipelined ----------
        with nc.named_scope("attn"), \
             tc.tile_pool(name="pep", bufs=3) as pep, \
             tc.tile_pool(name="ptp", bufs=3) as ptp, \
             tc.tile_pool(name="denp", bufs=4) as denp, \
             tc.tile_pool(name="outp", bufs=3) as outp, \
             tc.tile_pool(name="psS", bufs=2, space="PSUM") as psS, \
             tc.tile_pool(name="psT", bufs=2, space="PSUM") as psT, \
             tc.tile_pool(name="psV", bufs=2, space="PSUM") as psV:

            items = [(j, c) for j in range(NQ) for c in range(nchunks[j])]
            units = (["g0"] if causal else []) + items
            state = {}
            av_of = {}

            def emit_S(u):
                if u == "g0":
                    psg = psS.tile([P, P], F32, tag="s", name="psg0")
                    for do in range(DO):
                        nc.tensor.matmul(
                            psg, lhsT=qt0_sb[:, do, :], rhs=kt0_sb[:, do, :],
                            start=(do == 0), stop=(do == DO - 1))
                    nc.vector.tensor_tensor(out=psg, in0=psg, in1=cm0_sb, op=add)
                    peg = pep.tile([P, P], BF16, tag="p", name="peg0")
                    nc.scalar.activation(out=peg, in_=psg, func=Exp,
                                         scale=inv_s, accum_out=accg)
                    state[u] = peg
                    return
                j, c = u
                ps = psS.tile([P, 512], F32, tag="s", name=f"ps{c}_{j}")
                for ep in range(EP):
                    nc.tensor.matmul(
                        ps,
                        lhsT=qt_sb[:, 2 * ep:2 * ep + 2, j * P:(j + 1) * P],
                        rhs=kt_sb[:, 2 * ep:2 * ep + 2, c * 512:(c + 1) * 512],
                        start=(ep == 0), stop=(ep == EP - 1),
                        perf_mode=DR)
                if causal and c == nchunks[j] - 1:
                    p_j = SLOTS[j][1]
                    nc.vector.tensor_tensor(out=ps, in0=ps,
                                            in1=cm_sb[:, p_j, :], op=add)
                pe = pep.tile([P, 512], BF16, tag="p", name=f"pe{c}_{j}")
                nc.scalar.activation(out=pe, in_=ps, func=Exp,
                                     scale=exp_scale, bias=ebias_sb,
                                     accum_out=accs[j][:, c:c + 1])
                state[u] = pe

            def emit_T(u):
                pe = state[u]
                if u == "g0":
                    ptps = psT.tile([P, P], BF16, tag="t", name="ptg0")
                    nc.tensor.transpose(ptps, pe, identb)
                    pt = ptp.tile([P, P], BF16, tag="pt", name="ptg0_sb")
                    nc.vector.tensor_copy(out=pt, in_=ptps)
                else:
                    j, c = u
                    ptps = psT.tile([P, 4, P], BF16, tag="t", name=f"ptps{c}_{j}")
                    for t in range(4):
                        nc.tensor.transpose(ptps[:, t, :],
                                            pe[:, t * P:(t + 1) * P], identb)
                    pt = ptp.tile([P, 4, P], F8, tag="pt", name=f"pt{c}_{j}")
                    nc.vector.tensor_copy(out=pt, in_=ptps)
                state[u] = pt

            def emit_A(u):
                pt = state.pop(u)
                if u == "g0":
                    avg = psV.tile([P, D], F32, tag="avp", name="avg0")
                    for half in range(2):
                        nc.tensor.matmul(
                            avg[:, half * 512:(half + 1) * 512],
                            lhsT=pt,
                            rhs=v0_sb[:, half * 512:(half + 1) * 512],
                            start=True, stop=True)
                    recg = denp.tile([P, 1], F32, tag="rec", name="recg0")
                    nc.vector.reciprocal(out=recg, in_=accg)
                    og = outp.tile([P, D], F32, tag="o", name="og0")
                    nc.vector.scalar_tensor_tensor(
                        out=og, in0=avg, scalar=recg, in1=bv_sb,
                        op0=mult, op1=add)
                    nc.sync.dma_start(out=outg0, in_=og)
                    return
                j, c = u
                n_j = nchunks[j]
                if c == 0:
                    av_of[j] = psV.tile([P, D], F32, tag="avp", name=f"av{j}")
                av = av_of[j]
                for tp in range(2):
                    for half in range(2):
                        nc.tensor.matmul(
                            av[:, half * 512:(half + 1) * 512],
                            lhsT=pt[:, 2 * tp:2 * tp + 2, :],
                            rhs=v_sb[:, 4 * c + 2 * tp:4 * c + 2 * tp + 2,
                                     half * 512:(half + 1) * 512],
                            start=(c == 0 and tp == 0),
                            stop=(c == n_j - 1 and tp == 1),
                            perf_mode=DR)
                if c == n_j - 1:
                    den = denp.tile([P, 1], F32, tag="den", name=f"den{j}")
                    nc.vector.tensor_reduce(
                        out=den, in_=accs[j][:, 0:n_j],
                        axis=mybir.AxisListType.X, op=add)
                    den64 = denp.tile([P, 1], F32, tag="den64", name=f"den64{j}")
                    nc.vector.tensor_scalar_mul(den64, den, float(WV_SCALE * RTZ_C))
                    rec = denp.tile([P, 1], F32, tag="rec", name=f"rec{j}")
                    nc.vector.reciprocal(out=rec, in_=den64)
                    o = outp.tile([P, D], F32, tag="o", name=f"o{j}")
                    nc.vector.scalar_tensor_tensor(
                        out=o, in0=av_of.pop(j), scalar=rec, in1=bv_sb,
                        op0=mult, op1=add)
                    nc.sync.dma_start(out=out[j], in_=o)

            N = len(units)
            for t in range(N + 2):
                if t < N:
                    emit_S(units[t])
                if 1 <= t <= N:
                    emit_T(units[t - 1])
                if t >= 2:
                    emit_A(units[t - 2])

    nc.compile()
    return nc


def _get_program(causal: bool):
    key = bool(causal)
    if key not in _PROG_CACHE:
        _PROG_CACHE[key] = build_program(key)
    return _PROG_CACHE[key]


def _shard_inputs(encoded_q, encoded_k, encoded_v, W_q, b_q, W_k, b_k,
                  W_v, b_v, causal):
    """Build the per-core in_maps (all host-side numpy)."""
    def wlayout(W, scale, dt):
        return np.ascontiguousarray(
            (W.T * scale).reshape(EO, P, D).transpose(1, 0, 2)).astype(dt)

    wq8 = wlayout(W_q, WQK_SCALE, NP_F8)
    wk8 = wlayout(W_k, WQK_SCALE, NP_F8)
    wv8 = wlayout(W_v, WV_SCALE, NP_F8)
    bqh = np.ascontiguousarray(b_q.reshape(DO, P).T)
    bkh = np.ascontiguousarray(b_k.reshape(DO, P).T)
    bvh = np.ascontiguousarray(np.broadcast_to(b_v, (P, D)))
    if causal:
        wqb = wlayout(W_q, 1.0, NP_BF16)
        wkb = wlayout(W_k, 1.0, NP_BF16)
        wvb = wlayout(W_v, 1.0, NP_BF16)
        qi = np.arange(P)[:, None]
        cm0h = np.where(np.arange(P)[None, :] <= qi, 0.0, NEG).astype(np.float32)

    kf = np.arange(512)[None, :]
    in_maps = []
    for c in range(N_CORES):
        b, h = divmod(c, 2)
        gts = _slot_gtiles(h, causal)
        Xq = np.concatenate([encoded_q[b, g * P:(g + 1) * P, :] for g in gts], 0)
        xqh = np.ascontiguousarray(
            Xq.T.reshape(EO, P, 2, 512).transpose(2, 1, 0, 3)).astype(NP_F8)
        xkh = np.ascontiguousarray(
            encoded_k[b].T.reshape(EO, P, KC, 512).transpose(2, 1, 0, 3)
        ).astype(NP_F8)
        xvh = np.ascontiguousarray(
            encoded_v[b].T.reshape(EO, P, 16, P).transpose(2, 1, 0, 3)
        ).astype(NP_F8)
        cmh = np.zeros((P, 2, 512), np.float32)
        im = {
            "xq": xqh, "xk": xkh, "xv": xvh,
            "wq": wq8, "wk": wk8, "wv": wv8,
            "bq": bqh, "bk": bkh, "bv": bvh, "cm": cmh,
            "bq2": np.float32(RTZ_C) * bqh, "bk2": np.float32(RTZ_C) * bkh,
        }
        if causal:
            qi = np.arange(P)[:, None]
            for p in range(2):
                r = 2 * h + p
                cmh[:, p, :] = np.where(kf <= r * P + qi, 0.0, NEG)
            def x0layout(X):
                return np.ascontiguousarray(
                    X[b, :P, :].T.reshape(EO, P, P).transpose(1, 0, 2)
                ).astype(NP_BF16)
            im.update({
                "xq0": x0layout(encoded_q), "xk0": x0layout(encoded_k),
                "xv0": x0layout(encoded_v),
                "wqb": wqb, "wkb": wkb, "wvb": wvb, "cm0": cm0h,
            })
        in_maps.append(im)
    return in_maps


def kernel(encoded_q, encoded_k, encoded_v, W_q, b_q, W_k, b_k, W_v, b_v,
           parameter_mask, _want_trace=False, _trace_dir=None):
    causal = bool(np.asarray(parameter_mask).item())
    encoded_q = np.asarray(encoded_q, np.float32)
    encoded_k = np.asarray(encoded_k, np.float32)
    encoded_v = np.asarray(encoded_v, np.float32)
    nc = _get_program(causal)
    in_maps = _shard_inputs(encoded_q, encoded_k, encoded_v,
                            np.asarray(W_q, np.float32), np.asarray(b_q, np.float32),
                            np.asarray(W_k, np.float32), np.asarray(b_k, np.float32),
                            np.asarray(W_v, np.float32), np.asarray(b_v, np.float32),
                            causal)
    kw = {}
    if _want_trace:
        kw = dict(trace=True, tmpdir=_trace_dir)
    res = run_bass_kernel_spmd(nc, in_maps, core_ids=list(range(N_CORES)), **kw)

    full = np.empty((B, S, D), np.float32)
    for c in range(N_CORES):
        b, h = divmod(c, 2)
        o = res.results[c]["out"]
        for j, g in enumerate(_slot_gtiles(h, causal)):
            full[b, g * P:(g + 1) * P, :] = o[j]
        if causal and h == 0:
            full[b, 0:P, :] = res.results[c]["outg0"]
    if _want_trace:
        return full, res
    return full
